# revision 21
# baseline (speedup 1.0000x reference)
"""Trainium2 Bass kernel for nn_BaselineProt (embedding_lookup).

The reference computes, per drug-pair sample:
    multihot(drug) @ W0.T  ==  sum of W0 columns at the drug's (deduped)
    target proteins -- i.e. an embedding-table gather/sum, followed by a
    tiny MLP tower on each leg and a dot product between the two legs.

Structure (8 NeuronCores, data-parallel):
  Launch A: drugs sharded 500/core (padded to 512). Each core issues 32
      dma_gathers (512 rows each, round-robin over the 4 SWDGE queues
      for drain parallelism) of 512B bf16 rows of the transposed W0
      table (dups remapped to a zero row so `.set`-style multihot
      semantics are preserved). Per-gather partial trees + per-sub-batch
      combines reduce into an E-table shard [512, 256] while later
      gathers drain. HW-measured: the gather drain is per-descriptor
      latency-bound (~45ns/desc/engine at 4 queues); sorting, fp8, and
      bigger gathers do not move it, so this launch sits at that wall.
  Host:     concatenates the 8 E shards into E [4096, 256] (pure data
      movement) and replicates it to every core.
  Launch B: batch sharded 1024 samples/core. The per-sample cell+bias
      rows are computed by a one-hot matmul on the otherwise-idle PE
      during the Q7 library-load shadow (cells are only 32 rows, so no
      gather needed). Eight 256-idx transpose-mode gather waves pull
      E[d0], E[d1] per sample in feature-major layout (transpose mode
      requires single_packet=True, which caps waves at ~64 descriptors
      per engine); contiguous DVE add + relu form h0; two matmul layers
      (W1, W2) and a ones-matmul pair-dot produce the [1024] outputs.
"""

import os

os.environ.setdefault("JAX_PLATFORMS", "")

import numpy as np
import ml_dtypes

import concourse.bacc as bacc
import concourse.mybir as mybir
from concourse.tile import TileContext
from concourse import library_config
from concourse.bass_utils import run_bass_kernel_spmd

# Problem constants (hardcoded per harness contract).
B = 8192            # samples
P = 19000           # proteins
C = 32              # cell lines
D = 4000            # drugs
T = 32              # targets per drug
F = 256             # first hidden dim
H1 = 128            # second hidden dim
H2 = 64             # output dim per tower

NCORES = 8
DRUGS_PER_CORE = D // NCORES          # 500
DRUGS_PAD = 512                       # per-core padded drug count
SAMPLES_PER_CORE = B // NCORES        # 1024
ZROW = P + C                          # zero row in the W0T table (19032)
TAB_ROWS = ZROW + 8                   # pad table rows to 19040
E_ROWS = NCORES * DRUGS_PAD           # 4096 rows of E
EXT_ROWS = E_ROWS + C                 # + 32 cellb0 rows = 4128
NI_A = DRUGS_PAD * T                  # 16384 gather idxs per core, launch A
NI_B = 2 * SAMPLES_PER_CORE           # 2048 gather idxs per core, launch B
N_SUB = 4                             # launch A sub-batches (128 drugs each)
NG_A = 32                             # launch A gathers (512 idxs each)
SP_A = False                          # launch A single_packet
# transpose gathers need single_packet=True, which coalesces each engine's
# descriptor stream into ONE packet (<=64 descs) -> keep waves small
NG_B = 8                              # launch B gather waves (256 idxs each)
NQ = 4                                # SWDGE queues

_BF16 = mybir.dt.bfloat16
_F32 = mybir.dt.float32
_I16 = mybir.dt.int16

_cache = {}


def _wrap_idx(flat):
    """Flat gather order -> the [128, n/16] int16 SBUF layout dma_gather
    expects (idx i at partition i%16, slot i//16; replicated to all 8 Q7
    core slices)."""
    n = flat.shape[0]
    assert n % 16 == 0
    arr = flat.astype(np.int16).reshape(n // 16, 16).T.copy()
    return np.tile(arr, (8, 1))


def _build_kernel_a():
    nc = bacc.Bacc("TRN2", target_bir_lowering=True, num_swdge_queues=NQ)
    tab = nc.dram_tensor("tab", [TAB_ROWS, F], _BF16, kind="ExternalInput")
    idxs = nc.dram_tensor("idxs", [128, NI_A // 16], _I16, kind="ExternalInput")
    e_out = nc.dram_tensor("e_out", [DRUGS_PAD, F], _BF16, kind="ExternalOutput")

    ni_s = NI_A // NG_A                           # idxs per gather
    per_sub = NG_A // N_SUB                       # gathers per sub-batch
    slots = T // per_sub                          # t-slots per gather
    with TileContext(nc) as tc:
        nc.gpsimd.load_library(library_config.mlp)
        with (
            tc.tile_pool(name="idx", bufs=1) as ip,
            tc.tile_pool(name="g", bufs=1) as gp,
        ):
            idx_t = ip.tile([128, NI_A // 16], _I16)
            nc.sync.dma_start(out=idx_t[:, :], in_=idxs[:, :])
            # issue all gathers up front, round-robin over the 4 SWDGE
            # queues, so Q7 descriptor-gen stays ahead of the SDMA drain
            gs = []
            for b in range(N_SUB):
                g = gp.tile([128, T, F], _BF16, tag=f"g{b}")
                for h in range(per_sub):
                    s = b * per_sub + h
                    nc.gpsimd.dma_gather(
                        g[:, h * slots:(h + 1) * slots, :],
                        tab[:],
                        idx_t[:, s * (ni_s // 16):(s + 1) * (ni_s // 16)],
                        ni_s, ni_s, F,
                        single_packet=SP_A, queue_num=s % NQ,
                    )
                gs.append(g)
            # per-gather partial trees (each starts as soon as ITS gather
            # drains) + per-sub-batch combine, so only the last gather's
            # partial + combine is exposed at the tail
            assert slots == 4
            for b in range(N_SUB):
                g = gs[b]
                for h in range(per_sub):
                    base = h * slots
                    nc.vector.tensor_tensor(
                        out=g[:, base:base + 2, :],
                        in0=g[:, base:base + 2, :],
                        in1=g[:, base + 2:base + 4, :],
                        op=mybir.AluOpType.add,
                    )
                    nc.vector.tensor_tensor(
                        out=g[:, base:base + 1, :],
                        in0=g[:, base:base + 1, :],
                        in1=g[:, base + 1:base + 2, :],
                        op=mybir.AluOpType.add,
                    )
                    if h > 0:
                        # incremental accumulate, interleaved with the
                        # partials so each add fires right after its
                        # gather's partial; only the last one is exposed
                        nc.vector.tensor_tensor(
                            out=g[:, 0:1, :],
                            in0=g[:, 0:1, :],
                            in1=g[:, base:base + 1, :],
                            op=mybir.AluOpType.add,
                        )
                nc.sync.dma_start(
                    out=e_out[b * 128:(b + 1) * 128, :], in_=g[:, 0, :]
                )
    nc.compile()
    return nc


def _build_kernel_b():
    nc = bacc.Bacc("TRN2", target_bir_lowering=True, num_swdge_queues=NQ)
    # E table arrives host-permuted: partition-major [128, 32*F] so one
    # contiguous line-rate HWDGE load stages it into SBUF; row r lives at
    # partition r%128, rank r//128 -- exactly dma_gather's SBUF-source
    # token layout, so gather idx values are still plain row ids
    etab = nc.dram_tensor("etab", [128, (E_ROWS // 128) * F], _BF16,
                          kind="ExternalInput")
    idxs = nc.dram_tensor("idxs", [128, NI_B // 16], _I16, kind="ExternalInput")
    w1t = nc.dram_tensor("w1t", [F, H1], _BF16, kind="ExternalInput")
    w2t = nc.dram_tensor("w2t", [H1, H2], _BF16, kind="ExternalInput")
    cbt = nc.dram_tensor("cbt", [C, F], _BF16, kind="ExternalInput")
    oht = nc.dram_tensor("oht", [C, SAMPLES_PER_CORE], _BF16,
                         kind="ExternalInput")
    b1t = nc.dram_tensor("b1t", [128, 1], _F32, kind="ExternalInput")
    b2t = nc.dram_tensor("b2t", [64, 1], _F32, kind="ExternalInput")
    y = nc.dram_tensor("y", [1, SAMPLES_PER_CORE], _F32, kind="ExternalOutput")

    S = SAMPLES_PER_CORE                      # 1024
    L = 2 * S                                 # 2048 legs
    NT = 4                                    # matmul N tiles of 512
    TN = L // NT                              # 512
    SN = TN // 2                              # 256 samples per tile
    ni_g = NI_B // NG_B                       # 256 idxs per gather wave
    sg = ni_g // 2                            # 128 samples per wave
    with TileContext(nc) as tc:
        nc.gpsimd.load_library(library_config.mlp)
        with (
            tc.tile_pool(name="const", bufs=1) as cp,
            tc.tile_pool(name="act", bufs=1) as ap,
            tc.tile_pool(name="psc", bufs=1, space="PSUM") as pc,
            tc.tile_pool(name="ps1p", bufs=4, space="PSUM") as pp1,
            tc.tile_pool(name="ps", bufs=2, space="PSUM") as pp,
            tc.tile_pool(name="ps3p", bufs=1, space="PSUM") as pp3,
        ):
            # stage E into SBUF (2MB, line rate) on the ACT HWDGE ring so
            # the small SP-ring loads below run in parallel; finishes well
            # inside the Q7 library-load shadow
            e_sb = cp.tile([128, E_ROWS // 128, F], _BF16, tag="esb")
            nc.scalar.dma_start(
                out=e_sb[:, :, :],
                in_=etab.ap().rearrange("p (a f) -> p a f", a=E_ROWS // 128),
            )
            idx_t = cp.tile([128, NI_B // 16], _I16)
            nc.sync.dma_start(out=idx_t[:, :], in_=idxs[:, :])
            # W1T is [256, H1]; SBUF partition dim is 128 -> [128, 2, H1]
            w1_t = cp.tile([128, 2, H1], _BF16, tag="w1")
            nc.sync.dma_start(
                out=w1_t[:, :, :],
                in_=w1t.ap().rearrange("(c p) h -> p c h", p=128),
            )
            w2_t = cp.tile([128, H2], _BF16, tag="w2")
            nc.sync.dma_start(out=w2_t[:, :], in_=w2t[:, :])
            # cellb0 rows (W0cell + b0, host-folded): [32, 2 fblocks, 128]
            cb_t = cp.tile([C, 2, 128], _BF16, tag="cb")
            nc.sync.dma_start(
                out=cb_t[:, :, :],
                in_=cbt.ap().rearrange("c (b f) -> c b f", b=2),
            )
            oh_t = cp.tile([C, S], _BF16, tag="oh")
            nc.sync.dma_start(out=oh_t[:, :], in_=oht[:, :])
            b1_t = cp.tile([128, 1], _F32, tag="b1")
            nc.sync.dma_start(out=b1_t[:, :], in_=b1t[:, :])
            b2_t = cp.tile([64, 1], _F32, tag="b2")
            nc.sync.dma_start(out=b2_t[:, :], in_=b2t[:, :])
            ones = cp.tile([64, 1], _F32, tag="ones")
            nc.vector.memset(ones[:, :], 1.0)

            # per-sample cell+bias rows via one-hot matmul on the idle PE
            # (runs in the shadow of the Q7 library load); duplicated per
            # leg on the way out of PSUM: cells2[:, fb, 2s+leg] = cell[s]
            cells2 = ap.tile([128, 2, L], _BF16, tag="cells2")
            for fb in range(2):
                for q in range(2):
                    psc = pc.tile([128, TN], _F32, tag="psc")
                    nc.tensor.matmul(
                        psc[:, :], cb_t[:, fb, :], oh_t[:, q * TN:(q + 1) * TN],
                        start=True, stop=True,
                    )
                    for leg in range(2):
                        nc.vector.tensor_copy(
                            cells2[:, fb, 2 * q * TN + leg:2 * (q + 1) * TN:2],
                            psc[:, :],
                        )

            # E-row gather: per sample s, rows E[d0], E[d1] at columns
            # 2s, 2s+1; feature-major via transpose mode, SBUF-source so
            # the drain pays no per-descriptor HBM read latency
            xts = []
            for g in range(NG_B):
                xt = ap.tile([128, 2, ni_g], _BF16, tag=f"xt{g}")
                nc.gpsimd.dma_gather(
                    xt[:, :, :], e_sb[:, :, :],
                    idx_t[:, g * (ni_g // 16):(g + 1) * (ni_g // 16)],
                    ni_g, ni_g, F,
                    # transpose-mode gathers corrupt with single_packet=False
                    transpose=True, single_packet=True, queue_num=g % NQ,
                    sbuf_tokens_per_rank=128,
                    sbuf_free_dim_per_rank=F * 2,
                )
                xts.append(xt)

            # h0 = relu(E_leg + cellb0[s]) on DVE, bf16, contiguous ops
            h0 = ap.tile([128, 2, L], _BF16, tag="h0")
            for g in range(NG_B):
                nc.vector.tensor_tensor(
                    out=h0[:, :, g * ni_g:(g + 1) * ni_g],
                    in0=xts[g][:, :, :],
                    in1=cells2[:, :, g * ni_g:(g + 1) * ni_g],
                    op=mybir.AluOpType.add,
                )
            # relu in place, one op per matmul N-tile (512 legs = 2 waves)
            for k in range(NT):
                nc.vector.tensor_scalar_max(
                    h0[:, :, k * TN:(k + 1) * TN],
                    h0[:, :, k * TN:(k + 1) * TN],
                    0.0,
                )

            h1 = ap.tile([128, L], _BF16, tag="h1")
            h2 = ap.tile([64, L], _F32, tag="h2")
            prod = ap.tile([64, S], _F32, tag="prod")
            out_sb = ap.tile([1, S], _F32, tag="out")
            # layer 1 as two stationary-amortized sweeps (LS of W1 chunk
            # once per c, not once per tile); 4 PSUM banks live at once
            ps1s = [pp1.tile([128, TN], _F32, tag="ps1", name=f"ps1_{i}")
                    for i in range(NT)]
            for c in range(2):
                for nt in range(NT):
                    nc.tensor.matmul(
                        ps1s[nt][:, :], w1_t[:, c, :],
                        h0[:, c, nt * TN:(nt + 1) * TN],
                        start=(c == 0), stop=(c == 1),
                        skip_group_check=True,
                    )
            for nt in range(NT):
                nc.scalar.activation(
                    h1[:, nt * TN:(nt + 1) * TN], ps1s[nt][:, :],
                    mybir.ActivationFunctionType.Relu,
                    bias=b1_t[:, 0:1], scale=1.0,
                )
            ps2s = []
            for nt in range(NT):
                ps2 = pp.tile([64, TN], _F32, tag="ps2")
                nc.tensor.matmul(
                    ps2[:, :], w2_t[:, :], h1[:, nt * TN:(nt + 1) * TN],
                    start=True, stop=True,
                )
                nc.scalar.activation(
                    h2[:, nt * TN:(nt + 1) * TN], ps2[:, :],
                    mybir.ActivationFunctionType.Identity,
                    bias=b2_t[:, 0:1], scale=1.0,
                )
                # pair product for this tile's 256 samples
                nc.vector.tensor_tensor(
                    out=prod[:, nt * SN:(nt + 1) * SN],
                    in0=h2[:, nt * TN:(nt + 1) * TN:2],
                    in1=h2[:, nt * TN + 1:(nt + 1) * TN:2],
                    op=mybir.AluOpType.mult,
                )
            for nt in range(NT):
                ps3 = pp3.tile([1, SN], _F32, tag="ps3")
                nc.tensor.matmul(
                    ps3[:, :], ones[:, :], prod[:, nt * SN:(nt + 1) * SN],
                    start=True, stop=True,
                )
                nc.vector.tensor_copy(
                    out_sb[:, nt * SN:(nt + 1) * SN], ps3[:, :]
                )
            nc.sync.dma_start(out=y[:, :], in_=out_sb[:, :])
    nc.compile()
    return nc


def _get_kernels():
    if "a" not in _cache:
        _cache["a"] = _build_kernel_a()
    if "b" not in _cache:
        _cache["b"] = _build_kernel_b()
    return _cache["a"], _cache["b"]


def _prep(drug_pairs, cell_lines, drug_targets, W0, b0, W1, b1, W2, b2):
    """Host-side data layout: shard, transpose, cast, build gather indices."""
    dt = np.asarray(drug_targets, dtype=np.int64)                  # [D, T]
    # dedup per row (reference uses .set -> dup targets count once)
    dup = (dt[:, :, None] == dt[:, None, :]) & (
        np.arange(T)[None, :, None] > np.arange(T)[None, None, :]
    )
    idx = np.where(dup.any(-1), ZROW, dt).astype(np.int32)          # [D, T]

    # W0T table: [P+C rows, F] bf16 + zero row + pad
    w0t = np.zeros((TAB_ROWS, F), dtype=ml_dtypes.bfloat16)
    w0t[: P + C] = np.asarray(W0, np.float32).T.astype(ml_dtypes.bfloat16)

    # launch A per-core gather index arrays
    idx_a = []
    for c in range(NCORES):
        rows = np.full((DRUGS_PAD, T), ZROW, np.int32)
        rows[:DRUGS_PER_CORE] = idx[c * DRUGS_PER_CORE:(c + 1) * DRUGS_PER_CORE]
        # flat j = b*4096 + t*128 + p  ->  drug 128b+p, target t
        flat = rows.reshape(4, 128, T).transpose(0, 2, 1).reshape(-1)
        idx_a.append(_wrap_idx(flat))

    # launch B per-core index arrays (E rows per leg) + cell one-hots
    dp = np.asarray(drug_pairs, dtype=np.int64)                     # [B, 2]
    cl = np.asarray(cell_lines, dtype=np.int64)                     # [B]
    e_row = (dp // DRUGS_PER_CORE) * DRUGS_PAD + (dp % DRUGS_PER_CORE)
    idx_b, oh_b = [], []
    for c in range(NCORES):
        sl = slice(c * SAMPLES_PER_CORE, (c + 1) * SAMPLES_PER_CORE)
        idx_b.append(_wrap_idx(e_row[sl].reshape(-1)))
        oh = np.zeros((C, SAMPLES_PER_CORE), dtype=ml_dtypes.bfloat16)
        oh[cl[sl], np.arange(SAMPLES_PER_CORE)] = 1.0
        oh_b.append(oh)

    w1t = np.ascontiguousarray(
        np.asarray(W1, np.float32).T.astype(ml_dtypes.bfloat16))    # [F, H1]
    w2t = np.ascontiguousarray(
        np.asarray(W2, np.float32).T.astype(ml_dtypes.bfloat16))    # [H1, H2]
    b1t = np.asarray(b1, np.float32).reshape(128, 1).copy()
    b2t = np.asarray(b2, np.float32).reshape(64, 1).copy()
    # cell rows with b0 folded in: relu input is E[d] + (W0cell[:,c] + b0)
    celltab = (
        np.asarray(W0, np.float32)[:, P:P + C].T + np.asarray(b0, np.float32)
    ).astype(ml_dtypes.bfloat16)                                    # [C, F]
    return w0t, idx_a, idx_b, oh_b, w1t, w2t, b1t, b2t, celltab


def _run(inputs, trace=False):
    nca, ncb = _get_kernels()
    w0t, idx_a, idx_b, oh_b, w1t, w2t, b1t, b2t, celltab = _prep(**inputs)

    in_a = [{"tab": w0t, "idxs": idx_a[c]} for c in range(NCORES)]
    res_a = run_bass_kernel_spmd(
        nca, in_a, core_ids=list(range(NCORES)), trace=trace)

    e_ext = np.concatenate(
        [res_a.results[c]["e_out"] for c in range(NCORES)], axis=0
    )
    assert e_ext.shape == (E_ROWS, F)
    # permute to the SBUF-source token layout: row r -> partition r%128,
    # rank r//128 (pure data movement)
    e_perm = np.ascontiguousarray(
        e_ext.reshape(E_ROWS // 128, 128, F).transpose(1, 0, 2)
    ).reshape(128, (E_ROWS // 128) * F)

    in_b = [
        {"etab": e_perm, "idxs": idx_b[c], "w1t": w1t, "w2t": w2t,
         "cbt": celltab, "oht": oh_b[c], "b1t": b1t, "b2t": b2t}
        for c in range(NCORES)
    ]
    res_b = run_bass_kernel_spmd(
        ncb, in_b, core_ids=list(range(NCORES)), trace=trace)

    out = np.concatenate(
        [res_b.results[c]["y"].reshape(-1) for c in range(NCORES)]
    ).astype(np.float32)
    times = (res_a.exec_time_ns, res_b.exec_time_ns)
    return out, times


def kernel(**inputs) -> np.ndarray:
    out, _ = _run(inputs, trace=False)
    return out


# revision 26
# speedup vs baseline: 1.1041x; 1.1041x over previous
"""Trainium2 Bass kernel for nn_BaselineProt (embedding_lookup).

The reference computes, per drug-pair sample:
    multihot(drug) @ W0.T  ==  sum of W0 columns at the drug's (deduped)
    target proteins -- i.e. an embedding-table gather/sum, followed by a
    tiny MLP tower on each leg and a dot product between the two legs.

Structure (8 NeuronCores, data-parallel):
  Launch A: drugs sharded 500/core (padded to 512). Each core issues 32
      dma_gathers (512 rows each, round-robin over the 4 SWDGE queues
      for drain parallelism) of 512B bf16 rows of the transposed W0
      table (dups remapped to a zero row so `.set`-style multihot
      semantics are preserved). Per-gather partial trees + per-sub-batch
      combines reduce into an E-table shard [512, 256] while later
      gathers drain. HW-measured: the gather drain is per-descriptor
      latency-bound (~45ns/desc/engine at 4 queues); sorting, fp8, and
      bigger gathers do not move it, so this launch sits at that wall.
  Host:     concatenates the 8 E shards into E [4096, 256] (pure data
      movement) and replicates it to every core.
  Launch B: batch sharded 1024 samples/core. The per-sample cell+bias
      rows are computed by a one-hot matmul on the otherwise-idle PE
      during the Q7 library-load shadow (cells are only 32 rows, so no
      gather needed). Eight 256-idx transpose-mode gather waves pull
      E[d0], E[d1] per sample in feature-major layout (transpose mode
      requires single_packet=True, which caps waves at ~64 descriptors
      per engine); contiguous DVE add + relu form h0; two matmul layers
      (W1, W2) and a ones-matmul pair-dot produce the [1024] outputs.
"""

import os

os.environ.setdefault("JAX_PLATFORMS", "")

import numpy as np
import ml_dtypes

import concourse.bacc as bacc
import concourse.mybir as mybir
from concourse.tile import TileContext
from concourse import library_config
from concourse.bass_utils import run_bass_kernel_spmd

# Problem constants (hardcoded per harness contract).
B = 8192            # samples
P = 19000           # proteins
C = 32              # cell lines
D = 4000            # drugs
T = 32              # targets per drug
F = 256             # first hidden dim
H1 = 128            # second hidden dim
H2 = 64             # output dim per tower

NCORES = 8
DRUGS_PER_CORE = D // NCORES          # 500
DRUGS_PAD = 512                       # per-core padded drug count
SAMPLES_PER_CORE = B // NCORES        # 1024
ZROW = P + C                          # zero row in the W0T table (19032)
TAB_ROWS = ZROW + 8                   # pad table rows to 19040
E_ROWS = NCORES * DRUGS_PAD           # 4096 rows of E
EXT_ROWS = E_ROWS + C                 # + 32 cellb0 rows = 4128
NI_A = DRUGS_PAD * T                  # 16384 gather idxs per core, launch A
NI_B = 2 * SAMPLES_PER_CORE           # 2048 gather idxs per core, launch B
N_SUB = 4                             # launch A sub-batches (128 drugs each)
NG_A = 32                             # launch A gathers (512 idxs each)
SP_A = False                          # launch A single_packet
# transpose gathers need single_packet=True, which coalesces each engine's
# descriptor stream into ONE packet (<=64 descs) -> keep waves small
NG_B = 8                              # launch B gather waves (256 idxs each)
NQ = 4                                # SWDGE queues

_BF16 = mybir.dt.bfloat16
_F32 = mybir.dt.float32
_I16 = mybir.dt.int16

_cache = {}


def _wrap_idx(flat):
    """Flat gather order -> the [128, n/16] int16 SBUF layout dma_gather
    expects (idx i at partition i%16, slot i//16; replicated to all 8 Q7
    core slices)."""
    n = flat.shape[0]
    assert n % 16 == 0
    arr = flat.astype(np.int16).reshape(n // 16, 16).T.copy()
    return np.tile(arr, (8, 1))


def _build_kernel_a():
    nc = bacc.Bacc("TRN2", target_bir_lowering=True, num_swdge_queues=NQ)
    tab = nc.dram_tensor("tab", [TAB_ROWS, F], _BF16, kind="ExternalInput")
    idxs = nc.dram_tensor("idxs", [128, NI_A // 16], _I16, kind="ExternalInput")
    e_out = nc.dram_tensor("e_out", [DRUGS_PAD, F], _BF16, kind="ExternalOutput")

    ni_s = NI_A // NG_A                           # idxs per gather
    per_sub = NG_A // N_SUB                       # gathers per sub-batch
    slots = T // per_sub                          # t-slots per gather
    with TileContext(nc) as tc:
        nc.gpsimd.load_library(library_config.mlp)
        with (
            tc.tile_pool(name="idx", bufs=1) as ip,
            tc.tile_pool(name="g", bufs=1) as gp,
        ):
            idx_t = ip.tile([128, NI_A // 16], _I16)
            nc.sync.dma_start(out=idx_t[:, :], in_=idxs[:, :])
            # issue all gathers up front, round-robin over the 4 SWDGE
            # queues, so Q7 descriptor-gen stays ahead of the SDMA drain
            gs = []
            for b in range(N_SUB):
                g = gp.tile([128, T, F], _BF16, tag=f"g{b}")
                for h in range(per_sub):
                    s = b * per_sub + h
                    nc.gpsimd.dma_gather(
                        g[:, h * slots:(h + 1) * slots, :],
                        tab[:],
                        idx_t[:, s * (ni_s // 16):(s + 1) * (ni_s // 16)],
                        ni_s, ni_s, F,
                        single_packet=SP_A, queue_num=s % NQ,
                    )
                gs.append(g)
            # per-gather partial trees (each starts as soon as ITS gather
            # drains) + per-sub-batch combine, so only the last gather's
            # partial + combine is exposed at the tail
            assert slots == 4
            for b in range(N_SUB):
                g = gs[b]
                for h in range(per_sub):
                    base = h * slots
                    nc.vector.tensor_tensor(
                        out=g[:, base:base + 2, :],
                        in0=g[:, base:base + 2, :],
                        in1=g[:, base + 2:base + 4, :],
                        op=mybir.AluOpType.add,
                    )
                    nc.vector.tensor_tensor(
                        out=g[:, base:base + 1, :],
                        in0=g[:, base:base + 1, :],
                        in1=g[:, base + 1:base + 2, :],
                        op=mybir.AluOpType.add,
                    )
                    if h > 0:
                        # incremental accumulate, interleaved with the
                        # partials so each add fires right after its
                        # gather's partial; only the last one is exposed
                        nc.vector.tensor_tensor(
                            out=g[:, 0:1, :],
                            in0=g[:, 0:1, :],
                            in1=g[:, base:base + 1, :],
                            op=mybir.AluOpType.add,
                        )
                nc.sync.dma_start(
                    out=e_out[b * 128:(b + 1) * 128, :], in_=g[:, 0, :]
                )
    nc.compile()
    return nc


def _build_kernel_b():
    nc = bacc.Bacc("TRN2", target_bir_lowering=True, num_swdge_queues=NQ)
    # NOTE: an SBUF-source (staged-E) variant of the gather was measured
    # 14us SLOWER than gathering straight from HBM -- transpose-mode
    # SBUF->SBUF DMA hits the xbar/SBUF-DMA serialization hazard
    etab = nc.dram_tensor("etab", [E_ROWS, F], _BF16, kind="ExternalInput")
    idxs = nc.dram_tensor("idxs", [128, NI_B // 16], _I16, kind="ExternalInput")
    w1t = nc.dram_tensor("w1t", [F, H1], _BF16, kind="ExternalInput")
    w2t = nc.dram_tensor("w2t", [H1, H2], _BF16, kind="ExternalInput")
    cbt = nc.dram_tensor("cbt", [C, F], _BF16, kind="ExternalInput")
    oht = nc.dram_tensor("oht", [C, SAMPLES_PER_CORE], _BF16,
                         kind="ExternalInput")
    b1t = nc.dram_tensor("b1t", [128, 1], _F32, kind="ExternalInput")
    b2t = nc.dram_tensor("b2t", [64, 1], _F32, kind="ExternalInput")
    y = nc.dram_tensor("y", [1, SAMPLES_PER_CORE], _F32, kind="ExternalOutput")

    S = SAMPLES_PER_CORE                      # 1024
    L = 2 * S                                 # 2048 legs
    NT = 4                                    # matmul N tiles of 512
    TN = L // NT                              # 512
    SN = TN // 2                              # 256 samples per tile
    ni_g = NI_B // NG_B                       # 256 idxs per gather wave
    sg = ni_g // 2                            # 128 samples per wave
    with TileContext(nc) as tc:
        nc.gpsimd.load_library(library_config.mlp)
        with (
            tc.tile_pool(name="const", bufs=1) as cp,
            tc.tile_pool(name="act", bufs=1) as ap,
            tc.tile_pool(name="psc", bufs=1, space="PSUM") as pc,
            tc.tile_pool(name="ps1p", bufs=4, space="PSUM") as pp1,
            tc.tile_pool(name="ps", bufs=2, space="PSUM") as pp,
            tc.tile_pool(name="ps3p", bufs=1, space="PSUM") as pp3,
        ):
            idx_t = cp.tile([128, NI_B // 16], _I16)
            nc.sync.dma_start(out=idx_t[:, :], in_=idxs[:, :])
            # W1T is [256, H1]; SBUF partition dim is 128 -> [128, 2, H1]
            w1_t = cp.tile([128, 2, H1], _BF16, tag="w1")
            nc.sync.dma_start(
                out=w1_t[:, :, :],
                in_=w1t.ap().rearrange("(c p) h -> p c h", p=128),
            )
            w2_t = cp.tile([128, H2], _BF16, tag="w2")
            nc.sync.dma_start(out=w2_t[:, :], in_=w2t[:, :])
            # cellb0 rows (W0cell + b0, host-folded): [32, 2 fblocks, 128]
            cb_t = cp.tile([C, 2, 128], _BF16, tag="cb")
            nc.sync.dma_start(
                out=cb_t[:, :, :],
                in_=cbt.ap().rearrange("c (b f) -> c b f", b=2),
            )
            oh_t = cp.tile([C, S], _BF16, tag="oh")
            nc.sync.dma_start(out=oh_t[:, :], in_=oht[:, :])
            b1_t = cp.tile([128, 1], _F32, tag="b1")
            nc.sync.dma_start(out=b1_t[:, :], in_=b1t[:, :])
            b2_t = cp.tile([64, 1], _F32, tag="b2")
            nc.sync.dma_start(out=b2_t[:, :], in_=b2t[:, :])
            ones = cp.tile([64, 1], _F32, tag="ones")
            nc.vector.memset(ones[:, :], 1.0)

            # per-sample cell+bias rows via one-hot matmul on the idle PE
            # (runs in the shadow of the Q7 library load); duplicated per
            # leg on the way out of PSUM: cells2[:, fb, 2s+leg] = cell[s]
            cells2 = ap.tile([128, 2, L], _BF16, tag="cells2")
            for fb in range(2):
                for q in range(2):
                    psc = pc.tile([128, TN], _F32, tag="psc")
                    nc.tensor.matmul(
                        psc[:, :], cb_t[:, fb, :], oh_t[:, q * TN:(q + 1) * TN],
                        start=True, stop=True,
                    )
                    for leg in range(2):
                        nc.vector.tensor_copy(
                            cells2[:, fb, 2 * q * TN + leg:2 * (q + 1) * TN:2],
                            psc[:, :],
                        )

            # E-row gather: per sample s, rows E[d0], E[d1] at columns
            # 2s, 2s+1; feature-major via transpose mode.
            xts = []
            for g in range(NG_B):
                xt = ap.tile([128, 2, ni_g], _BF16, tag=f"xt{g}")
                nc.gpsimd.dma_gather(
                    xt[:, :, :], etab[:],
                    idx_t[:, g * (ni_g // 16):(g + 1) * (ni_g // 16)],
                    ni_g, ni_g, F,
                    # transpose-mode gathers corrupt with single_packet=False
                    transpose=True, single_packet=True, queue_num=g % NQ,
                )
                xts.append(xt)

            # h0 = relu(E_leg + cellb0[s]) on DVE, bf16, contiguous ops
            h0 = ap.tile([128, 2, L], _BF16, tag="h0")
            for g in range(NG_B):
                nc.vector.tensor_tensor(
                    out=h0[:, :, g * ni_g:(g + 1) * ni_g],
                    in0=xts[g][:, :, :],
                    in1=cells2[:, :, g * ni_g:(g + 1) * ni_g],
                    op=mybir.AluOpType.add,
                )
            # relu in place, one op per matmul N-tile (512 legs = 2 waves)
            for k in range(NT):
                nc.vector.tensor_scalar_max(
                    h0[:, :, k * TN:(k + 1) * TN],
                    h0[:, :, k * TN:(k + 1) * TN],
                    0.0,
                )

            h1 = ap.tile([128, L], _BF16, tag="h1")
            h2 = ap.tile([64, L], _F32, tag="h2")
            prod = ap.tile([64, S], _F32, tag="prod")
            out_sb = ap.tile([1, S], _F32, tag="out")
            # layer 1 as two stationary-amortized sweeps (LS of W1 chunk
            # once per c, not once per tile); 4 PSUM banks live at once
            ps1s = [pp1.tile([128, TN], _F32, tag="ps1", name=f"ps1_{i}")
                    for i in range(NT)]
            for c in range(2):
                for nt in range(NT):
                    nc.tensor.matmul(
                        ps1s[nt][:, :], w1_t[:, c, :],
                        h0[:, c, nt * TN:(nt + 1) * TN],
                        start=(c == 0), stop=(c == 1),
                        skip_group_check=True,
                    )
            for nt in range(NT):
                nc.scalar.activation(
                    h1[:, nt * TN:(nt + 1) * TN], ps1s[nt][:, :],
                    mybir.ActivationFunctionType.Relu,
                    bias=b1_t[:, 0:1], scale=1.0,
                )
            ps2s = []
            for nt in range(NT):
                ps2 = pp.tile([64, TN], _F32, tag="ps2")
                nc.tensor.matmul(
                    ps2[:, :], w2_t[:, :], h1[:, nt * TN:(nt + 1) * TN],
                    start=True, stop=True,
                )
                # bias-add + PSUM->SBUF on DVE: keeps the h2->prod chain
                # on one engine (no scalar handoff)
                nc.vector.tensor_scalar_add(
                    h2[:, nt * TN:(nt + 1) * TN], ps2[:, :], b2_t[:, 0:1],
                )
                # pair product for this tile's 256 samples
                nc.vector.tensor_tensor(
                    out=prod[:, nt * SN:(nt + 1) * SN],
                    in0=h2[:, nt * TN:(nt + 1) * TN:2],
                    in1=h2[:, nt * TN + 1:(nt + 1) * TN:2],
                    op=mybir.AluOpType.mult,
                )
            for nt in range(NT):
                ps3 = pp3.tile([1, SN], _F32, tag="ps3")
                nc.tensor.matmul(
                    ps3[:, :], ones[:, :], prod[:, nt * SN:(nt + 1) * SN],
                    start=True, stop=True,
                )
                nc.vector.tensor_copy(
                    out_sb[:, nt * SN:(nt + 1) * SN], ps3[:, :]
                )
            nc.sync.dma_start(out=y[:, :], in_=out_sb[:, :])
    nc.compile()
    return nc


def _get_kernels():
    if "a" not in _cache:
        _cache["a"] = _build_kernel_a()
    if "b" not in _cache:
        _cache["b"] = _build_kernel_b()
    return _cache["a"], _cache["b"]


def _prep(drug_pairs, cell_lines, drug_targets, W0, b0, W1, b1, W2, b2):
    """Host-side data layout: shard, transpose, cast, build gather indices."""
    dt = np.asarray(drug_targets, dtype=np.int64)                  # [D, T]
    # dedup per row (reference uses .set -> dup targets count once)
    dup = (dt[:, :, None] == dt[:, None, :]) & (
        np.arange(T)[None, :, None] > np.arange(T)[None, None, :]
    )
    idx = np.where(dup.any(-1), ZROW, dt).astype(np.int32)          # [D, T]

    # W0T table: [P+C rows, F] bf16 + zero row + pad
    w0t = np.zeros((TAB_ROWS, F), dtype=ml_dtypes.bfloat16)
    w0t[: P + C] = np.asarray(W0, np.float32).T.astype(ml_dtypes.bfloat16)

    # launch A per-core gather index arrays
    idx_a = []
    for c in range(NCORES):
        rows = np.full((DRUGS_PAD, T), ZROW, np.int32)
        rows[:DRUGS_PER_CORE] = idx[c * DRUGS_PER_CORE:(c + 1) * DRUGS_PER_CORE]
        # flat j = b*4096 + t*128 + p  ->  drug 128b+p, target t
        flat = rows.reshape(4, 128, T).transpose(0, 2, 1).reshape(-1)
        idx_a.append(_wrap_idx(flat))

    # launch B per-core index arrays (E rows per leg) + cell one-hots
    dp = np.asarray(drug_pairs, dtype=np.int64)                     # [B, 2]
    cl = np.asarray(cell_lines, dtype=np.int64)                     # [B]
    e_row = (dp // DRUGS_PER_CORE) * DRUGS_PAD + (dp % DRUGS_PER_CORE)
    idx_b, oh_b = [], []
    for c in range(NCORES):
        sl = slice(c * SAMPLES_PER_CORE, (c + 1) * SAMPLES_PER_CORE)
        idx_b.append(_wrap_idx(e_row[sl].reshape(-1)))
        oh = np.zeros((C, SAMPLES_PER_CORE), dtype=ml_dtypes.bfloat16)
        oh[cl[sl], np.arange(SAMPLES_PER_CORE)] = 1.0
        oh_b.append(oh)

    w1t = np.ascontiguousarray(
        np.asarray(W1, np.float32).T.astype(ml_dtypes.bfloat16))    # [F, H1]
    w2t = np.ascontiguousarray(
        np.asarray(W2, np.float32).T.astype(ml_dtypes.bfloat16))    # [H1, H2]
    b1t = np.asarray(b1, np.float32).reshape(128, 1).copy()
    b2t = np.asarray(b2, np.float32).reshape(64, 1).copy()
    # cell rows with b0 folded in: relu input is E[d] + (W0cell[:,c] + b0)
    celltab = (
        np.asarray(W0, np.float32)[:, P:P + C].T + np.asarray(b0, np.float32)
    ).astype(ml_dtypes.bfloat16)                                    # [C, F]
    return w0t, idx_a, idx_b, oh_b, w1t, w2t, b1t, b2t, celltab


def _run(inputs, trace=False):
    nca, ncb = _get_kernels()
    w0t, idx_a, idx_b, oh_b, w1t, w2t, b1t, b2t, celltab = _prep(**inputs)

    in_a = [{"tab": w0t, "idxs": idx_a[c]} for c in range(NCORES)]
    res_a = run_bass_kernel_spmd(
        nca, in_a, core_ids=list(range(NCORES)), trace=trace)

    e_ext = np.concatenate(
        [res_a.results[c]["e_out"] for c in range(NCORES)], axis=0
    )
    assert e_ext.shape == (E_ROWS, F)

    in_b = [
        {"etab": e_ext, "idxs": idx_b[c], "w1t": w1t, "w2t": w2t,
         "cbt": celltab, "oht": oh_b[c], "b1t": b1t, "b2t": b2t}
        for c in range(NCORES)
    ]
    res_b = run_bass_kernel_spmd(
        ncb, in_b, core_ids=list(range(NCORES)), trace=trace)

    out = np.concatenate(
        [res_b.results[c]["y"].reshape(-1) for c in range(NCORES)]
    ).astype(np.float32)
    times = (res_a.exec_time_ns, res_b.exec_time_ns)
    return out, times


def kernel(**inputs) -> np.ndarray:
    out, _ = _run(inputs, trace=False)
    return out


# revision 27
# speedup vs baseline: 1.1104x; 1.0057x over previous
"""Trainium2 Bass kernel for nn_BaselineProt (embedding_lookup).

The reference computes, per drug-pair sample:
    multihot(drug) @ W0.T  ==  sum of W0 columns at the drug's (deduped)
    target proteins -- i.e. an embedding-table gather/sum, followed by a
    tiny MLP tower on each leg and a dot product between the two legs.

Structure (8 NeuronCores, data-parallel):
  Launch A: drugs sharded 500/core (padded to 512). Each core issues 32
      dma_gathers (512 rows each, round-robin over the 4 SWDGE queues
      for drain parallelism) of 512B bf16 rows of the transposed W0
      table (dups remapped to a zero row so `.set`-style multihot
      semantics are preserved). Per-gather partial trees + per-sub-batch
      combines reduce into an E-table shard [512, 256] while later
      gathers drain. HW-measured: the gather drain is per-descriptor
      latency-bound (~45ns/desc/engine at 4 queues); sorting, fp8, and
      bigger gathers do not move it, so this launch sits at that wall.
  Host:     concatenates the 8 E shards into E [4096, 256] (pure data
      movement) and replicates it to every core.
  Launch B: batch sharded 1024 samples/core. The per-sample cell+bias
      rows are computed by a one-hot matmul on the otherwise-idle PE
      during the Q7 library-load shadow (cells are only 32 rows, so no
      gather needed). Eight 256-idx transpose-mode gather waves pull
      E[d0], E[d1] per sample in feature-major layout (transpose mode
      requires single_packet=True, which caps waves at ~64 descriptors
      per engine); contiguous DVE add + relu form h0; two matmul layers
      (W1, W2) and a ones-matmul pair-dot produce the [1024] outputs.
"""

import os

os.environ.setdefault("JAX_PLATFORMS", "")

import numpy as np
import ml_dtypes

import concourse.bacc as bacc
import concourse.mybir as mybir
from concourse.tile import TileContext
from concourse import library_config
from concourse.bass_utils import run_bass_kernel_spmd

# Problem constants (hardcoded per harness contract).
B = 8192            # samples
P = 19000           # proteins
C = 32              # cell lines
D = 4000            # drugs
T = 32              # targets per drug
F = 256             # first hidden dim
H1 = 128            # second hidden dim
H2 = 64             # output dim per tower

NCORES = 8
DRUGS_PER_CORE = D // NCORES          # 500
DRUGS_PAD = 512                       # per-core padded drug count
SAMPLES_PER_CORE = B // NCORES        # 1024
ZROW = P + C                          # zero row in the W0T table (19032)
TAB_ROWS = ZROW + 8                   # pad table rows to 19040
E_ROWS = NCORES * DRUGS_PAD           # 4096 rows of E
EXT_ROWS = E_ROWS + C                 # + 32 cellb0 rows = 4128
NI_A = DRUGS_PAD * T                  # 16384 gather idxs per core, launch A
NI_B = 2 * SAMPLES_PER_CORE           # 2048 gather idxs per core, launch B
N_SUB = 4                             # launch A sub-batches (128 drugs each)
NG_A = 32                             # launch A gathers (512 idxs each)
SP_A = False                          # launch A single_packet
# transpose gathers need single_packet=True, which coalesces each engine's
# descriptor stream into ONE packet (<=64 descs) -> keep waves small
NG_B = 8                              # launch B gather waves (256 idxs each)
NQ = 4                                # SWDGE queues

_BF16 = mybir.dt.bfloat16
_F32 = mybir.dt.float32
_I16 = mybir.dt.int16

_cache = {}


def _wrap_idx(flat):
    """Flat gather order -> the [128, n/16] int16 SBUF layout dma_gather
    expects (idx i at partition i%16, slot i//16; replicated to all 8 Q7
    core slices)."""
    n = flat.shape[0]
    assert n % 16 == 0
    arr = flat.astype(np.int16).reshape(n // 16, 16).T.copy()
    return np.tile(arr, (8, 1))


def _build_kernel_a():
    nc = bacc.Bacc("TRN2", target_bir_lowering=True, num_swdge_queues=NQ)
    tab = nc.dram_tensor("tab", [TAB_ROWS, F], _BF16, kind="ExternalInput")
    idxs = nc.dram_tensor("idxs", [128, NI_A // 16], _I16, kind="ExternalInput")
    e_out = nc.dram_tensor("e_out", [DRUGS_PAD, F], _BF16, kind="ExternalOutput")

    ni_s = NI_A // NG_A                           # idxs per gather
    per_sub = NG_A // N_SUB                       # gathers per sub-batch
    slots = T // per_sub                          # t-slots per gather
    with TileContext(nc) as tc:
        nc.gpsimd.load_library(library_config.mlp)
        with (
            tc.tile_pool(name="idx", bufs=1) as ip,
            tc.tile_pool(name="g", bufs=1) as gp,
        ):
            idx_t = ip.tile([128, NI_A // 16], _I16)
            nc.sync.dma_start(out=idx_t[:, :], in_=idxs[:, :])
            # issue all gathers up front, round-robin over the 4 SWDGE
            # queues, so Q7 descriptor-gen stays ahead of the SDMA drain
            gs = []
            for b in range(N_SUB):
                g = gp.tile([128, T, F], _BF16, tag=f"g{b}")
                for h in range(per_sub):
                    s = b * per_sub + h
                    nc.gpsimd.dma_gather(
                        g[:, h * slots:(h + 1) * slots, :],
                        tab[:],
                        idx_t[:, s * (ni_s // 16):(s + 1) * (ni_s // 16)],
                        ni_s, ni_s, F,
                        single_packet=SP_A, queue_num=s % NQ,
                    )
                gs.append(g)
            # per-gather partial trees (each starts as soon as ITS gather
            # drains) + per-sub-batch combine, so only the last gather's
            # partial + combine is exposed at the tail
            assert slots == 4
            for b in range(N_SUB):
                g = gs[b]
                for h in range(per_sub):
                    base = h * slots
                    nc.vector.tensor_tensor(
                        out=g[:, base:base + 2, :],
                        in0=g[:, base:base + 2, :],
                        in1=g[:, base + 2:base + 4, :],
                        op=mybir.AluOpType.add,
                    )
                    nc.vector.tensor_tensor(
                        out=g[:, base:base + 1, :],
                        in0=g[:, base:base + 1, :],
                        in1=g[:, base + 1:base + 2, :],
                        op=mybir.AluOpType.add,
                    )
                    if h > 0:
                        # incremental accumulate, interleaved with the
                        # partials so each add fires right after its
                        # gather's partial; only the last one is exposed
                        nc.vector.tensor_tensor(
                            out=g[:, 0:1, :],
                            in0=g[:, 0:1, :],
                            in1=g[:, base:base + 1, :],
                            op=mybir.AluOpType.add,
                        )
                nc.sync.dma_start(
                    out=e_out[b * 128:(b + 1) * 128, :], in_=g[:, 0, :]
                )
    nc.compile()
    return nc


def _build_kernel_b():
    nc = bacc.Bacc("TRN2", target_bir_lowering=True, num_swdge_queues=NQ)
    # NOTE: an SBUF-source (staged-E) variant of the gather was measured
    # 14us SLOWER than gathering straight from HBM -- transpose-mode
    # SBUF->SBUF DMA hits the xbar/SBUF-DMA serialization hazard
    etab = nc.dram_tensor("etab", [E_ROWS, F], _BF16, kind="ExternalInput")
    idxs = nc.dram_tensor("idxs", [128, NI_B // 16], _I16, kind="ExternalInput")
    w1t = nc.dram_tensor("w1t", [F, H1], _BF16, kind="ExternalInput")
    w2t = nc.dram_tensor("w2t", [H1, H2], _BF16, kind="ExternalInput")
    cbt = nc.dram_tensor("cbt", [C, F], _BF16, kind="ExternalInput")
    oht = nc.dram_tensor("oht", [C, SAMPLES_PER_CORE], _BF16,
                         kind="ExternalInput")
    b1t = nc.dram_tensor("b1t", [128, 1], _F32, kind="ExternalInput")
    b2t = nc.dram_tensor("b2t", [64, 1], _F32, kind="ExternalInput")
    y = nc.dram_tensor("y", [1, SAMPLES_PER_CORE], _F32, kind="ExternalOutput")

    S = SAMPLES_PER_CORE                      # 1024
    L = 2 * S                                 # 2048 legs
    NT = 4                                    # matmul N tiles of 512
    TN = L // NT                              # 512
    SN = TN // 2                              # 256 samples per tile
    ni_g = NI_B // NG_B                       # 256 idxs per gather wave
    sg = ni_g // 2                            # 128 samples per wave
    with TileContext(nc) as tc:
        nc.gpsimd.load_library(library_config.mlp)
        with (
            tc.tile_pool(name="const", bufs=1) as cp,
            tc.tile_pool(name="act", bufs=1) as ap,
            tc.tile_pool(name="psc", bufs=1, space="PSUM") as pc,
            tc.tile_pool(name="ps1p", bufs=4, space="PSUM") as pp1,
            tc.tile_pool(name="ps", bufs=2, space="PSUM") as pp,
            tc.tile_pool(name="ps3p", bufs=1, space="PSUM") as pp3,
        ):
            idx_t = cp.tile([128, NI_B // 16], _I16)
            nc.sync.dma_start(out=idx_t[:, :], in_=idxs[:, :])
            # W1T is [256, H1]; SBUF partition dim is 128 -> [128, 2, H1]
            w1_t = cp.tile([128, 2, H1], _BF16, tag="w1")
            nc.sync.dma_start(
                out=w1_t[:, :, :],
                in_=w1t.ap().rearrange("(c p) h -> p c h", p=128),
            )
            w2_t = cp.tile([128, H2], _BF16, tag="w2")
            nc.sync.dma_start(out=w2_t[:, :], in_=w2t[:, :])
            # cellb0 rows (W0cell + b0, host-folded): [32, 2 fblocks, 128]
            cb_t = cp.tile([C, 2, 128], _BF16, tag="cb")
            nc.sync.dma_start(
                out=cb_t[:, :, :],
                in_=cbt.ap().rearrange("c (b f) -> c b f", b=2),
            )
            oh_t = cp.tile([C, S], _BF16, tag="oh")
            nc.sync.dma_start(out=oh_t[:, :], in_=oht[:, :])
            b1_t = cp.tile([128, 1], _F32, tag="b1")
            nc.sync.dma_start(out=b1_t[:, :], in_=b1t[:, :])
            b2_t = cp.tile([64, 1], _F32, tag="b2")
            nc.sync.dma_start(out=b2_t[:, :], in_=b2t[:, :])
            ones = cp.tile([64, 1], _F32, tag="ones")
            nc.vector.memset(ones[:, :], 1.0)

            # per-sample cell+bias rows via one-hot matmul on the idle PE
            # (runs in the shadow of the Q7 library load); duplicated per
            # leg on the way out of PSUM: cells2[:, fb, 2s+leg] = cell[s]
            cells2 = ap.tile([128, 2, L], _BF16, tag="cells2")
            for fb in range(2):
                for q in range(2):
                    psc = pc.tile([128, TN], _F32, tag="psc")
                    nc.tensor.matmul(
                        psc[:, :], cb_t[:, fb, :], oh_t[:, q * TN:(q + 1) * TN],
                        start=True, stop=True,
                    )
                    for leg in range(2):
                        nc.vector.tensor_copy(
                            cells2[:, fb, 2 * q * TN + leg:2 * (q + 1) * TN:2],
                            psc[:, :],
                        )

            # E-row gather: per sample s, rows E[d0], E[d1] at columns
            # 2s, 2s+1; feature-major via transpose mode.
            xts = []
            for g in range(NG_B):
                xt = ap.tile([128, 2, ni_g], _BF16, tag=f"xt{g}")
                nc.gpsimd.dma_gather(
                    xt[:, :, :], etab[:],
                    idx_t[:, g * (ni_g // 16):(g + 1) * (ni_g // 16)],
                    ni_g, ni_g, F,
                    # transpose-mode gathers corrupt with single_packet=False
                    transpose=True, single_packet=True, queue_num=g % NQ,
                )
                xts.append(xt)

            # h0 = relu(E_leg + cellb0[s]) on DVE, bf16, contiguous ops
            h0 = ap.tile([128, 2, L], _BF16, tag="h0")
            for g in range(NG_B):
                nc.vector.tensor_tensor(
                    out=h0[:, :, g * ni_g:(g + 1) * ni_g],
                    in0=xts[g][:, :, :],
                    in1=cells2[:, :, g * ni_g:(g + 1) * ni_g],
                    op=mybir.AluOpType.add,
                )
            # relu in place, one op per matmul N-tile (512 legs = 2 waves)
            for k in range(NT):
                nc.vector.tensor_scalar_max(
                    h0[:, :, k * TN:(k + 1) * TN],
                    h0[:, :, k * TN:(k + 1) * TN],
                    0.0,
                )

            h1 = ap.tile([128, L], _BF16, tag="h1")
            h2 = ap.tile([64, L], _F32, tag="h2")
            prod = ap.tile([64, S], _F32, tag="prod")
            out_sb = ap.tile([1, S], _F32, tag="out")
            # layer 1 as two stationary-amortized sweeps (LS of W1 chunk
            # once per c, not once per tile); 4 PSUM banks live at once
            ps1s = [pp1.tile([128, TN], _F32, tag="ps1", name=f"ps1_{i}")
                    for i in range(NT)]
            for c in range(2):
                for nt in range(NT):
                    nc.tensor.matmul(
                        ps1s[nt][:, :], w1_t[:, c, :],
                        h0[:, c, nt * TN:(nt + 1) * TN],
                        start=(c == 0), stop=(c == 1),
                        skip_group_check=True,
                    )
            for nt in range(NT):
                nc.scalar.activation(
                    h1[:, nt * TN:(nt + 1) * TN], ps1s[nt][:, :],
                    mybir.ActivationFunctionType.Relu,
                    bias=b1_t[:, 0:1], scale=1.0,
                )
            ps2s = []
            for nt in range(NT):
                ps2 = pp.tile([64, TN], _F32, tag="ps2")
                nc.tensor.matmul(
                    ps2[:, :], w2_t[:, :], h1[:, nt * TN:(nt + 1) * TN],
                    start=True, stop=True,
                )
                nc.scalar.activation(
                    h2[:, nt * TN:(nt + 1) * TN], ps2[:, :],
                    mybir.ActivationFunctionType.Identity,
                    bias=b2_t[:, 0:1], scale=1.0,
                )
                # pair product for this tile's 256 samples
                nc.vector.tensor_tensor(
                    out=prod[:, nt * SN:(nt + 1) * SN],
                    in0=h2[:, nt * TN:(nt + 1) * TN:2],
                    in1=h2[:, nt * TN + 1:(nt + 1) * TN:2],
                    op=mybir.AluOpType.mult,
                )
            for nt in range(NT):
                ps3 = pp3.tile([1, SN], _F32, tag="ps3")
                nc.tensor.matmul(
                    ps3[:, :], ones[:, :], prod[:, nt * SN:(nt + 1) * SN],
                    start=True, stop=True,
                )
                nc.vector.tensor_copy(
                    out_sb[:, nt * SN:(nt + 1) * SN], ps3[:, :]
                )
            nc.sync.dma_start(out=y[:, :], in_=out_sb[:, :])
    nc.compile()
    return nc


def _get_kernels():
    if "a" not in _cache:
        _cache["a"] = _build_kernel_a()
    if "b" not in _cache:
        _cache["b"] = _build_kernel_b()
    return _cache["a"], _cache["b"]


def _prep(drug_pairs, cell_lines, drug_targets, W0, b0, W1, b1, W2, b2):
    """Host-side data layout: shard, transpose, cast, build gather indices."""
    dt = np.asarray(drug_targets, dtype=np.int64)                  # [D, T]
    # dedup per row (reference uses .set -> dup targets count once)
    dup = (dt[:, :, None] == dt[:, None, :]) & (
        np.arange(T)[None, :, None] > np.arange(T)[None, None, :]
    )
    idx = np.where(dup.any(-1), ZROW, dt).astype(np.int32)          # [D, T]

    # W0T table: [P+C rows, F] bf16 + zero row + pad
    w0t = np.zeros((TAB_ROWS, F), dtype=ml_dtypes.bfloat16)
    w0t[: P + C] = np.asarray(W0, np.float32).T.astype(ml_dtypes.bfloat16)

    # launch A per-core gather index arrays
    idx_a = []
    for c in range(NCORES):
        rows = np.full((DRUGS_PAD, T), ZROW, np.int32)
        rows[:DRUGS_PER_CORE] = idx[c * DRUGS_PER_CORE:(c + 1) * DRUGS_PER_CORE]
        # flat j = b*4096 + t*128 + p  ->  drug 128b+p, target t
        flat = rows.reshape(4, 128, T).transpose(0, 2, 1).reshape(-1)
        idx_a.append(_wrap_idx(flat))

    # launch B per-core index arrays (E rows per leg) + cell one-hots
    dp = np.asarray(drug_pairs, dtype=np.int64)                     # [B, 2]
    cl = np.asarray(cell_lines, dtype=np.int64)                     # [B]
    e_row = (dp // DRUGS_PER_CORE) * DRUGS_PAD + (dp % DRUGS_PER_CORE)
    idx_b, oh_b = [], []
    for c in range(NCORES):
        sl = slice(c * SAMPLES_PER_CORE, (c + 1) * SAMPLES_PER_CORE)
        idx_b.append(_wrap_idx(e_row[sl].reshape(-1)))
        oh = np.zeros((C, SAMPLES_PER_CORE), dtype=ml_dtypes.bfloat16)
        oh[cl[sl], np.arange(SAMPLES_PER_CORE)] = 1.0
        oh_b.append(oh)

    w1t = np.ascontiguousarray(
        np.asarray(W1, np.float32).T.astype(ml_dtypes.bfloat16))    # [F, H1]
    w2t = np.ascontiguousarray(
        np.asarray(W2, np.float32).T.astype(ml_dtypes.bfloat16))    # [H1, H2]
    b1t = np.asarray(b1, np.float32).reshape(128, 1).copy()
    b2t = np.asarray(b2, np.float32).reshape(64, 1).copy()
    # cell rows with b0 folded in: relu input is E[d] + (W0cell[:,c] + b0)
    celltab = (
        np.asarray(W0, np.float32)[:, P:P + C].T + np.asarray(b0, np.float32)
    ).astype(ml_dtypes.bfloat16)                                    # [C, F]
    return w0t, idx_a, idx_b, oh_b, w1t, w2t, b1t, b2t, celltab


def _run(inputs, trace=False):
    nca, ncb = _get_kernels()
    w0t, idx_a, idx_b, oh_b, w1t, w2t, b1t, b2t, celltab = _prep(**inputs)

    in_a = [{"tab": w0t, "idxs": idx_a[c]} for c in range(NCORES)]
    res_a = run_bass_kernel_spmd(
        nca, in_a, core_ids=list(range(NCORES)), trace=trace)

    e_ext = np.concatenate(
        [res_a.results[c]["e_out"] for c in range(NCORES)], axis=0
    )
    assert e_ext.shape == (E_ROWS, F)

    in_b = [
        {"etab": e_ext, "idxs": idx_b[c], "w1t": w1t, "w2t": w2t,
         "cbt": celltab, "oht": oh_b[c], "b1t": b1t, "b2t": b2t}
        for c in range(NCORES)
    ]
    res_b = run_bass_kernel_spmd(
        ncb, in_b, core_ids=list(range(NCORES)), trace=trace)

    out = np.concatenate(
        [res_b.results[c]["y"].reshape(-1) for c in range(NCORES)]
    ).astype(np.float32)
    times = (res_a.exec_time_ns, res_b.exec_time_ns)
    return out, times


def kernel(**inputs) -> np.ndarray:
    out, _ = _run(inputs, trace=False)
    return out


# revision 29
# speedup vs baseline: 1.1158x; 1.0048x over previous
"""Trainium2 Bass kernel for nn_BaselineProt (embedding_lookup).

The reference computes, per drug-pair sample:
    multihot(drug) @ W0.T  ==  sum of W0 columns at the drug's (deduped)
    target proteins -- i.e. an embedding-table gather/sum, followed by a
    tiny MLP tower on each leg and a dot product between the two legs.

Structure (8 NeuronCores, data-parallel):
  Launch A: drugs sharded 500/core (padded to 512). Each core issues 32
      dma_gathers (512 rows each, round-robin over the 4 SWDGE queues
      for drain parallelism) of 512B bf16 rows of the transposed W0
      table (dups remapped to a zero row so `.set`-style multihot
      semantics are preserved). Per-gather partial trees + per-sub-batch
      combines reduce into an E-table shard [512, 256] while later
      gathers drain. HW-measured: the gather drain is per-descriptor
      latency-bound (~45ns/desc/engine at 4 queues); sorting, fp8, and
      bigger gathers do not move it, so this launch sits at that wall.
  Host:     concatenates the 8 E shards into E [4096, 256] (pure data
      movement) and replicates it to every core.
  Launch B: batch sharded 1024 samples/core. The per-sample cell+bias
      rows are computed by a one-hot matmul on the otherwise-idle PE
      during the Q7 library-load shadow (cells are only 32 rows, so no
      gather needed). Eight 256-idx transpose-mode gather waves pull
      E[d0], E[d1] per sample in feature-major layout (transpose mode
      requires single_packet=True, which caps waves at ~64 descriptors
      per engine); contiguous DVE add + relu form h0; two matmul layers
      (W1, W2) and a ones-matmul pair-dot produce the [1024] outputs.
"""

import os

os.environ.setdefault("JAX_PLATFORMS", "")

import numpy as np
import ml_dtypes

import concourse.bacc as bacc
import concourse.mybir as mybir
from concourse.tile import TileContext
from concourse import library_config
from concourse.bass_utils import run_bass_kernel_spmd

# Problem constants (hardcoded per harness contract).
B = 8192            # samples
P = 19000           # proteins
C = 32              # cell lines
D = 4000            # drugs
T = 32              # targets per drug
F = 256             # first hidden dim
H1 = 128            # second hidden dim
H2 = 64             # output dim per tower

NCORES = 8
DRUGS_PER_CORE = D // NCORES          # 500
DRUGS_PAD = 512                       # per-core padded drug count
SAMPLES_PER_CORE = B // NCORES        # 1024
ZROW = P + C                          # zero row in the W0T table (19032)
TAB_ROWS = ZROW + 8                   # pad table rows to 19040
E_ROWS = NCORES * DRUGS_PAD           # 4096 rows of E
EXT_ROWS = E_ROWS + C                 # + 32 cellb0 rows = 4128
NI_A = DRUGS_PAD * T                  # 16384 gather idxs per core, launch A
NI_B = 2 * SAMPLES_PER_CORE           # 2048 gather idxs per core, launch B
N_SUB = 4                             # launch A sub-batches (128 drugs each)
NG_A = 32                             # launch A gathers (512 idxs each)
SP_A = False                          # launch A single_packet
# transpose gathers need single_packet=True, which coalesces each engine's
# descriptor stream into ONE packet (<=64 descs); 512-idx waves put the
# rx spray stream at exactly 64 descs/engine -- the ceiling, but it works
NG_B = 4                              # launch B gather waves (512 idxs each)
NQ = 4                                # SWDGE queues

_BF16 = mybir.dt.bfloat16
_F32 = mybir.dt.float32
_I16 = mybir.dt.int16

_cache = {}


def _wrap_idx(flat):
    """Flat gather order -> the [128, n/16] int16 SBUF layout dma_gather
    expects (idx i at partition i%16, slot i//16; replicated to all 8 Q7
    core slices)."""
    n = flat.shape[0]
    assert n % 16 == 0
    arr = flat.astype(np.int16).reshape(n // 16, 16).T.copy()
    return np.tile(arr, (8, 1))


def _build_kernel_a():
    nc = bacc.Bacc("TRN2", target_bir_lowering=True, num_swdge_queues=NQ)
    tab = nc.dram_tensor("tab", [TAB_ROWS, F], _BF16, kind="ExternalInput")
    idxs = nc.dram_tensor("idxs", [128, NI_A // 16], _I16, kind="ExternalInput")
    e_out = nc.dram_tensor("e_out", [DRUGS_PAD, F], _BF16, kind="ExternalOutput")

    ni_s = NI_A // NG_A                           # idxs per gather
    per_sub = NG_A // N_SUB                       # gathers per sub-batch
    slots = T // per_sub                          # t-slots per gather
    with TileContext(nc) as tc:
        nc.gpsimd.load_library(library_config.mlp)
        with (
            tc.tile_pool(name="idx", bufs=1) as ip,
            tc.tile_pool(name="g", bufs=1) as gp,
        ):
            idx_t = ip.tile([128, NI_A // 16], _I16)
            nc.sync.dma_start(out=idx_t[:, :], in_=idxs[:, :])
            # issue all gathers up front, round-robin over the 4 SWDGE
            # queues, so Q7 descriptor-gen stays ahead of the SDMA drain
            gs = []
            for b in range(N_SUB):
                g = gp.tile([128, T, F], _BF16, tag=f"g{b}")
                for h in range(per_sub):
                    s = b * per_sub + h
                    nc.gpsimd.dma_gather(
                        g[:, h * slots:(h + 1) * slots, :],
                        tab[:],
                        idx_t[:, s * (ni_s // 16):(s + 1) * (ni_s // 16)],
                        ni_s, ni_s, F,
                        single_packet=SP_A, queue_num=s % NQ,
                    )
                gs.append(g)
            # per-gather partial trees (each starts as soon as ITS gather
            # drains) + per-sub-batch combine, so only the last gather's
            # partial + combine is exposed at the tail
            assert slots == 4
            for b in range(N_SUB):
                g = gs[b]
                for h in range(per_sub):
                    base = h * slots
                    nc.vector.tensor_tensor(
                        out=g[:, base:base + 2, :],
                        in0=g[:, base:base + 2, :],
                        in1=g[:, base + 2:base + 4, :],
                        op=mybir.AluOpType.add,
                    )
                    nc.vector.tensor_tensor(
                        out=g[:, base:base + 1, :],
                        in0=g[:, base:base + 1, :],
                        in1=g[:, base + 1:base + 2, :],
                        op=mybir.AluOpType.add,
                    )
                    if h > 0:
                        # incremental accumulate, interleaved with the
                        # partials so each add fires right after its
                        # gather's partial; only the last one is exposed
                        nc.vector.tensor_tensor(
                            out=g[:, 0:1, :],
                            in0=g[:, 0:1, :],
                            in1=g[:, base:base + 1, :],
                            op=mybir.AluOpType.add,
                        )
                nc.sync.dma_start(
                    out=e_out[b * 128:(b + 1) * 128, :], in_=g[:, 0, :]
                )
    nc.compile()
    return nc


def _build_kernel_b():
    nc = bacc.Bacc("TRN2", target_bir_lowering=True, num_swdge_queues=NQ)
    # NOTE: an SBUF-source (staged-E) variant of the gather was measured
    # 14us SLOWER than gathering straight from HBM -- transpose-mode
    # SBUF->SBUF DMA hits the xbar/SBUF-DMA serialization hazard
    etab = nc.dram_tensor("etab", [E_ROWS, F], _BF16, kind="ExternalInput")
    idxs = nc.dram_tensor("idxs", [128, NI_B // 16], _I16, kind="ExternalInput")
    w1t = nc.dram_tensor("w1t", [F, H1], _BF16, kind="ExternalInput")
    w2t = nc.dram_tensor("w2t", [H1, H2], _BF16, kind="ExternalInput")
    cbt = nc.dram_tensor("cbt", [C, F], _BF16, kind="ExternalInput")
    oht = nc.dram_tensor("oht", [C, SAMPLES_PER_CORE], _BF16,
                         kind="ExternalInput")
    b1t = nc.dram_tensor("b1t", [128, 1], _F32, kind="ExternalInput")
    b2t = nc.dram_tensor("b2t", [64, 1], _F32, kind="ExternalInput")
    y = nc.dram_tensor("y", [1, SAMPLES_PER_CORE], _F32, kind="ExternalOutput")

    S = SAMPLES_PER_CORE                      # 1024
    L = 2 * S                                 # 2048 legs
    NT = 4                                    # matmul N tiles of 512
    TN = L // NT                              # 512
    SN = TN // 2                              # 256 samples per tile
    ni_g = NI_B // NG_B                       # 256 idxs per gather wave
    sg = ni_g // 2                            # 128 samples per wave
    with TileContext(nc) as tc:
        nc.gpsimd.load_library(library_config.mlp)
        with (
            tc.tile_pool(name="const", bufs=1) as cp,
            tc.tile_pool(name="act", bufs=1) as ap,
            tc.tile_pool(name="psc", bufs=1, space="PSUM") as pc,
            tc.tile_pool(name="ps1p", bufs=4, space="PSUM") as pp1,
            tc.tile_pool(name="ps", bufs=2, space="PSUM") as pp,
            tc.tile_pool(name="ps3p", bufs=1, space="PSUM") as pp3,
        ):
            idx_t = cp.tile([128, NI_B // 16], _I16)
            nc.sync.dma_start(out=idx_t[:, :], in_=idxs[:, :])
            # W1T is [256, H1]; SBUF partition dim is 128 -> [128, 2, H1]
            w1_t = cp.tile([128, 2, H1], _BF16, tag="w1")
            nc.sync.dma_start(
                out=w1_t[:, :, :],
                in_=w1t.ap().rearrange("(c p) h -> p c h", p=128),
            )
            w2_t = cp.tile([128, H2], _BF16, tag="w2")
            nc.sync.dma_start(out=w2_t[:, :], in_=w2t[:, :])
            # cellb0 rows (W0cell + b0, host-folded): [32, 2 fblocks, 128]
            cb_t = cp.tile([C, 2, 128], _BF16, tag="cb")
            nc.sync.dma_start(
                out=cb_t[:, :, :],
                in_=cbt.ap().rearrange("c (b f) -> c b f", b=2),
            )
            oh_t = cp.tile([C, S], _BF16, tag="oh")
            nc.sync.dma_start(out=oh_t[:, :], in_=oht[:, :])
            b1_t = cp.tile([128, 1], _F32, tag="b1")
            nc.sync.dma_start(out=b1_t[:, :], in_=b1t[:, :])
            b2_t = cp.tile([64, 1], _F32, tag="b2")
            nc.sync.dma_start(out=b2_t[:, :], in_=b2t[:, :])
            ones = cp.tile([64, 1], _F32, tag="ones")
            nc.vector.memset(ones[:, :], 1.0)

            # per-sample cell+bias rows via one-hot matmul on the idle PE
            # (runs in the shadow of the Q7 library load); duplicated per
            # leg on the way out of PSUM: cells2[:, fb, 2s+leg] = cell[s]
            cells2 = ap.tile([128, 2, L], _BF16, tag="cells2")
            for fb in range(2):
                for q in range(2):
                    psc = pc.tile([128, TN], _F32, tag="psc")
                    nc.tensor.matmul(
                        psc[:, :], cb_t[:, fb, :], oh_t[:, q * TN:(q + 1) * TN],
                        start=True, stop=True,
                    )
                    for leg in range(2):
                        nc.vector.tensor_copy(
                            cells2[:, fb, 2 * q * TN + leg:2 * (q + 1) * TN:2],
                            psc[:, :],
                        )

            # E-row gather: per sample s, rows E[d0], E[d1] at columns
            # 2s, 2s+1; feature-major via transpose mode.
            xts = []
            for g in range(NG_B):
                xt = ap.tile([128, 2, ni_g], _BF16, tag=f"xt{g}")
                nc.gpsimd.dma_gather(
                    xt[:, :, :], etab[:],
                    idx_t[:, g * (ni_g // 16):(g + 1) * (ni_g // 16)],
                    ni_g, ni_g, F,
                    # transpose-mode gathers corrupt with single_packet=False
                    transpose=True, single_packet=True, queue_num=g % NQ,
                )
                xts.append(xt)

            # h0 = relu(E_leg + cellb0[s]) on DVE, bf16, contiguous ops
            h0 = ap.tile([128, 2, L], _BF16, tag="h0")
            for g in range(NG_B):
                nc.vector.tensor_tensor(
                    out=h0[:, :, g * ni_g:(g + 1) * ni_g],
                    in0=xts[g][:, :, :],
                    in1=cells2[:, :, g * ni_g:(g + 1) * ni_g],
                    op=mybir.AluOpType.add,
                )
            # relu in place, one op per matmul N-tile (512 legs = 2 waves)
            for k in range(NT):
                nc.vector.tensor_scalar_max(
                    h0[:, :, k * TN:(k + 1) * TN],
                    h0[:, :, k * TN:(k + 1) * TN],
                    0.0,
                )

            h1 = ap.tile([128, L], _BF16, tag="h1")
            h2 = ap.tile([64, L], _F32, tag="h2")
            prod = ap.tile([64, S], _F32, tag="prod")
            out_sb = ap.tile([1, S], _F32, tag="out")
            # layer 1 as two stationary-amortized sweeps (LS of W1 chunk
            # once per c, not once per tile); 4 PSUM banks live at once
            ps1s = [pp1.tile([128, TN], _F32, tag="ps1", name=f"ps1_{i}")
                    for i in range(NT)]
            for c in range(2):
                for nt in range(NT):
                    nc.tensor.matmul(
                        ps1s[nt][:, :], w1_t[:, c, :],
                        h0[:, c, nt * TN:(nt + 1) * TN],
                        start=(c == 0), stop=(c == 1),
                        skip_group_check=True,
                    )
            for nt in range(NT):
                nc.scalar.activation(
                    h1[:, nt * TN:(nt + 1) * TN], ps1s[nt][:, :],
                    mybir.ActivationFunctionType.Relu,
                    bias=b1_t[:, 0:1], scale=1.0,
                )
            for nt in range(NT):
                ps2 = pp.tile([64, TN], _F32, tag="ps2")
                nc.tensor.matmul(
                    ps2[:, :], w2_t[:, :], h1[:, nt * TN:(nt + 1) * TN],
                    start=True, stop=True,
                )
                nc.scalar.activation(
                    h2[:, nt * TN:(nt + 1) * TN], ps2[:, :],
                    mybir.ActivationFunctionType.Identity,
                    bias=b2_t[:, 0:1], scale=1.0,
                )
                # pair product for this tile's 256 samples
                nc.vector.tensor_tensor(
                    out=prod[:, nt * SN:(nt + 1) * SN],
                    in0=h2[:, nt * TN:(nt + 1) * TN:2],
                    in1=h2[:, nt * TN + 1:(nt + 1) * TN:2],
                    op=mybir.AluOpType.mult,
                )
            for nt in range(NT):
                ps3 = pp3.tile([1, SN], _F32, tag="ps3")
                nc.tensor.matmul(
                    ps3[:, :], ones[:, :], prod[:, nt * SN:(nt + 1) * SN],
                    start=True, stop=True,
                )
                nc.vector.tensor_copy(
                    out_sb[:, nt * SN:(nt + 1) * SN], ps3[:, :]
                )
            nc.sync.dma_start(out=y[:, :], in_=out_sb[:, :])
    nc.compile()
    return nc


def _get_kernels():
    if "a" not in _cache:
        _cache["a"] = _build_kernel_a()
    if "b" not in _cache:
        _cache["b"] = _build_kernel_b()
    return _cache["a"], _cache["b"]


def _prep(drug_pairs, cell_lines, drug_targets, W0, b0, W1, b1, W2, b2):
    """Host-side data layout: shard, transpose, cast, build gather indices."""
    dt = np.asarray(drug_targets, dtype=np.int64)                  # [D, T]
    # dedup per row (reference uses .set -> dup targets count once)
    dup = (dt[:, :, None] == dt[:, None, :]) & (
        np.arange(T)[None, :, None] > np.arange(T)[None, None, :]
    )
    idx = np.where(dup.any(-1), ZROW, dt).astype(np.int32)          # [D, T]

    # W0T table: [P+C rows, F] bf16 + zero row + pad
    w0t = np.zeros((TAB_ROWS, F), dtype=ml_dtypes.bfloat16)
    w0t[: P + C] = np.asarray(W0, np.float32).T.astype(ml_dtypes.bfloat16)

    # launch A per-core gather index arrays
    idx_a = []
    for c in range(NCORES):
        rows = np.full((DRUGS_PAD, T), ZROW, np.int32)
        rows[:DRUGS_PER_CORE] = idx[c * DRUGS_PER_CORE:(c + 1) * DRUGS_PER_CORE]
        # flat j = b*4096 + t*128 + p  ->  drug 128b+p, target t
        flat = rows.reshape(4, 128, T).transpose(0, 2, 1).reshape(-1)
        idx_a.append(_wrap_idx(flat))

    # launch B per-core index arrays (E rows per leg) + cell one-hots
    dp = np.asarray(drug_pairs, dtype=np.int64)                     # [B, 2]
    cl = np.asarray(cell_lines, dtype=np.int64)                     # [B]
    e_row = (dp // DRUGS_PER_CORE) * DRUGS_PAD + (dp % DRUGS_PER_CORE)
    idx_b, oh_b = [], []
    for c in range(NCORES):
        sl = slice(c * SAMPLES_PER_CORE, (c + 1) * SAMPLES_PER_CORE)
        idx_b.append(_wrap_idx(e_row[sl].reshape(-1)))
        oh = np.zeros((C, SAMPLES_PER_CORE), dtype=ml_dtypes.bfloat16)
        oh[cl[sl], np.arange(SAMPLES_PER_CORE)] = 1.0
        oh_b.append(oh)

    w1t = np.ascontiguousarray(
        np.asarray(W1, np.float32).T.astype(ml_dtypes.bfloat16))    # [F, H1]
    w2t = np.ascontiguousarray(
        np.asarray(W2, np.float32).T.astype(ml_dtypes.bfloat16))    # [H1, H2]
    b1t = np.asarray(b1, np.float32).reshape(128, 1).copy()
    b2t = np.asarray(b2, np.float32).reshape(64, 1).copy()
    # cell rows with b0 folded in: relu input is E[d] + (W0cell[:,c] + b0)
    celltab = (
        np.asarray(W0, np.float32)[:, P:P + C].T + np.asarray(b0, np.float32)
    ).astype(ml_dtypes.bfloat16)                                    # [C, F]
    return w0t, idx_a, idx_b, oh_b, w1t, w2t, b1t, b2t, celltab


def _run(inputs, trace=False):
    nca, ncb = _get_kernels()
    w0t, idx_a, idx_b, oh_b, w1t, w2t, b1t, b2t, celltab = _prep(**inputs)

    in_a = [{"tab": w0t, "idxs": idx_a[c]} for c in range(NCORES)]
    res_a = run_bass_kernel_spmd(
        nca, in_a, core_ids=list(range(NCORES)), trace=trace)

    e_ext = np.concatenate(
        [res_a.results[c]["e_out"] for c in range(NCORES)], axis=0
    )
    assert e_ext.shape == (E_ROWS, F)

    in_b = [
        {"etab": e_ext, "idxs": idx_b[c], "w1t": w1t, "w2t": w2t,
         "cbt": celltab, "oht": oh_b[c], "b1t": b1t, "b2t": b2t}
        for c in range(NCORES)
    ]
    res_b = run_bass_kernel_spmd(
        ncb, in_b, core_ids=list(range(NCORES)), trace=trace)

    out = np.concatenate(
        [res_b.results[c]["y"].reshape(-1) for c in range(NCORES)]
    ).astype(np.float32)
    times = (res_a.exec_time_ns, res_b.exec_time_ns)
    return out, times


def kernel(**inputs) -> np.ndarray:
    out, _ = _run(inputs, trace=False)
    return out


# revision 30
# speedup vs baseline: 1.1435x; 1.0249x over previous
"""Trainium2 Bass kernel for nn_BaselineProt (embedding_lookup).

The reference computes, per drug-pair sample:
    multihot(drug) @ W0.T  ==  sum of W0 columns at the drug's (deduped)
    target proteins -- i.e. an embedding-table gather/sum, followed by a
    tiny MLP tower on each leg and a dot product between the two legs.

Structure (8 NeuronCores, data-parallel):
  Launch A: drugs sharded 500/core (padded to 512). Each core issues 32
      dma_gathers (512 rows each, round-robin over the 4 SWDGE queues
      for drain parallelism) of 512B bf16 rows of the transposed W0
      table (dups remapped to a zero row so `.set`-style multihot
      semantics are preserved). Per-gather partial trees + per-sub-batch
      combines reduce into an E-table shard [512, 256] while later
      gathers drain. HW-measured: the gather drain is per-descriptor
      latency-bound (~45ns/desc/engine at 4 queues); sorting, fp8, and
      bigger gathers do not move it, so this launch sits at that wall.
  Host:     concatenates the 8 E shards into E [4096, 256] (pure data
      movement) and replicates it to every core.
  Launch B: batch sharded 1024 samples/core. The per-sample cell+bias
      rows are computed by a one-hot matmul on the otherwise-idle PE
      during the Q7 library-load shadow (cells are only 32 rows, so no
      gather needed). Eight 256-idx transpose-mode gather waves pull
      E[d0], E[d1] per sample in feature-major layout (transpose mode
      requires single_packet=True, which caps waves at ~64 descriptors
      per engine); contiguous DVE add + relu form h0; two matmul layers
      (W1, W2) and a ones-matmul pair-dot produce the [1024] outputs.
"""

import os

os.environ.setdefault("JAX_PLATFORMS", "")

import numpy as np
import ml_dtypes

import concourse.bacc as bacc
import concourse.mybir as mybir
from concourse.tile import TileContext
from concourse import library_config
from concourse.bass_utils import run_bass_kernel_spmd

# Problem constants (hardcoded per harness contract).
B = 8192            # samples
P = 19000           # proteins
C = 32              # cell lines
D = 4000            # drugs
T = 32              # targets per drug
F = 256             # first hidden dim
H1 = 128            # second hidden dim
H2 = 64             # output dim per tower

NCORES = 8
DRUGS_PER_CORE = D // NCORES          # 500
DRUGS_PAD = 512                       # per-core padded drug count
SAMPLES_PER_CORE = B // NCORES        # 1024
ZROW = P + C                          # zero row in the W0T table (19032)
TAB_ROWS = ZROW + 8                   # pad table rows to 19040
E_ROWS = NCORES * DRUGS_PAD           # 4096 rows of E
EXT_ROWS = E_ROWS + C                 # + 32 cellb0 rows = 4128
NI_A = DRUGS_PAD * T                  # 16384 gather idxs per core, launch A
NI_B = 2 * SAMPLES_PER_CORE           # 2048 gather idxs per core, launch B
N_SUB = 4                             # launch A sub-batches (128 drugs each)
NG_A = 32                             # launch A gathers (512 idxs each)
SP_A = False                          # launch A single_packet
# transpose gathers need single_packet=True, which coalesces each engine's
# descriptor stream into ONE packet (<=64 descs); 512-idx waves put the
# rx spray stream at exactly 64 descs/engine -- the ceiling, but it works
NG_B = 4                              # launch B gather waves (512 idxs each)
NQ = 4                                # SWDGE queues

_BF16 = mybir.dt.bfloat16
_F32 = mybir.dt.float32
_I16 = mybir.dt.int16

_cache = {}


def _wrap_idx(flat):
    """Flat gather order -> the [128, n/16] int16 SBUF layout dma_gather
    expects (idx i at partition i%16, slot i//16; replicated to all 8 Q7
    core slices)."""
    n = flat.shape[0]
    assert n % 16 == 0
    arr = flat.astype(np.int16).reshape(n // 16, 16).T.copy()
    return np.tile(arr, (8, 1))


def _build_kernel_a():
    nc = bacc.Bacc("TRN2", target_bir_lowering=True, num_swdge_queues=NQ)
    tab = nc.dram_tensor("tab", [TAB_ROWS, F], _BF16, kind="ExternalInput")
    idxs = nc.dram_tensor("idxs", [128, NI_A // 16], _I16, kind="ExternalInput")
    e_out = nc.dram_tensor("e_out", [DRUGS_PAD, F], _BF16, kind="ExternalOutput")

    ni_s = NI_A // NG_A                           # idxs per gather
    per_sub = NG_A // N_SUB                       # gathers per sub-batch
    slots = T // per_sub                          # t-slots per gather
    with TileContext(nc) as tc:
        nc.gpsimd.load_library(library_config.mlp)
        with (
            tc.tile_pool(name="idx", bufs=1) as ip,
            tc.tile_pool(name="g", bufs=1) as gp,
        ):
            idx_t = ip.tile([128, NI_A // 16], _I16)
            nc.sync.dma_start(out=idx_t[:, :], in_=idxs[:, :])
            # issue all gathers up front, round-robin over the 4 SWDGE
            # queues, so Q7 descriptor-gen stays ahead of the SDMA drain
            gs = []
            for b in range(N_SUB):
                g = gp.tile([128, T, F], _BF16, tag=f"g{b}")
                for h in range(per_sub):
                    s = b * per_sub + h
                    nc.gpsimd.dma_gather(
                        g[:, h * slots:(h + 1) * slots, :],
                        tab[:],
                        idx_t[:, s * (ni_s // 16):(s + 1) * (ni_s // 16)],
                        ni_s, ni_s, F,
                        single_packet=SP_A, queue_num=s % NQ,
                    )
                gs.append(g)
            # per-gather partial trees (each starts as soon as ITS gather
            # drains) + per-sub-batch combine, so only the last gather's
            # partial + combine is exposed at the tail
            assert slots == 4
            for b in range(N_SUB):
                g = gs[b]
                for h in range(per_sub):
                    base = h * slots
                    nc.vector.tensor_tensor(
                        out=g[:, base:base + 2, :],
                        in0=g[:, base:base + 2, :],
                        in1=g[:, base + 2:base + 4, :],
                        op=mybir.AluOpType.add,
                    )
                    nc.vector.tensor_tensor(
                        out=g[:, base:base + 1, :],
                        in0=g[:, base:base + 1, :],
                        in1=g[:, base + 1:base + 2, :],
                        op=mybir.AluOpType.add,
                    )
                    if h > 0:
                        # incremental accumulate, interleaved with the
                        # partials so each add fires right after its
                        # gather's partial; only the last one is exposed
                        nc.vector.tensor_tensor(
                            out=g[:, 0:1, :],
                            in0=g[:, 0:1, :],
                            in1=g[:, base:base + 1, :],
                            op=mybir.AluOpType.add,
                        )
                nc.sync.dma_start(
                    out=e_out[b * 128:(b + 1) * 128, :], in_=g[:, 0, :]
                )
    nc.compile()
    return nc


def _build_kernel_b():
    nc = bacc.Bacc("TRN2", target_bir_lowering=True, num_swdge_queues=NQ)
    # NOTE: an SBUF-source (staged-E) variant of the gather was measured
    # 14us SLOWER than gathering straight from HBM -- transpose-mode
    # SBUF->SBUF DMA hits the xbar/SBUF-DMA serialization hazard
    etab = nc.dram_tensor("etab", [E_ROWS, F], _BF16, kind="ExternalInput")
    idxs = nc.dram_tensor("idxs", [128, NI_B // 16], _I16, kind="ExternalInput")
    w1t = nc.dram_tensor("w1t", [F, H1], _BF16, kind="ExternalInput")
    w2t = nc.dram_tensor("w2t", [H1, H2], _BF16, kind="ExternalInput")
    cbt = nc.dram_tensor("cbt", [C, F], _BF16, kind="ExternalInput")
    oht = nc.dram_tensor("oht", [C, SAMPLES_PER_CORE], _BF16,
                         kind="ExternalInput")
    b1t = nc.dram_tensor("b1t", [128, 1], _F32, kind="ExternalInput")
    b2t = nc.dram_tensor("b2t", [64, 1], _F32, kind="ExternalInput")
    y = nc.dram_tensor("y", [1, SAMPLES_PER_CORE], _F32, kind="ExternalOutput")

    S = SAMPLES_PER_CORE                      # 1024
    L = 2 * S                                 # 2048 legs
    NT = 4                                    # matmul N tiles of 512
    TN = L // NT                              # 512
    SN = TN // 2                              # 256 samples per tile
    ni_g = NI_B // NG_B                       # 256 idxs per gather wave
    sg = ni_g // 2                            # 128 samples per wave
    with TileContext(nc) as tc:
        nc.gpsimd.load_library(library_config.mlp)
        with (
            tc.tile_pool(name="const", bufs=1) as cp,
            tc.tile_pool(name="act", bufs=1) as ap,
            tc.tile_pool(name="psc", bufs=1, space="PSUM") as pc,
            tc.tile_pool(name="ps1p", bufs=4, space="PSUM") as pp1,
            tc.tile_pool(name="ps", bufs=2, space="PSUM") as pp,
            tc.tile_pool(name="ps3p", bufs=1, space="PSUM") as pp3,
        ):
            idx_t = cp.tile([128, NI_B // 16], _I16)
            nc.sync.dma_start(out=idx_t[:, :], in_=idxs[:, :])
            # W1T is [256, H1]; SBUF partition dim is 128 -> [128, 2, H1]
            w1_t = cp.tile([128, 2, H1], _BF16, tag="w1")
            nc.sync.dma_start(
                out=w1_t[:, :, :],
                in_=w1t.ap().rearrange("(c p) h -> p c h", p=128),
            )
            w2_t = cp.tile([128, H2], _BF16, tag="w2")
            nc.sync.dma_start(out=w2_t[:, :], in_=w2t[:, :])
            # cellb0 rows (W0cell + b0, host-folded): [32, 2 fblocks, 128]
            cb_t = cp.tile([C, 2, 128], _BF16, tag="cb")
            nc.sync.dma_start(
                out=cb_t[:, :, :],
                in_=cbt.ap().rearrange("c (b f) -> c b f", b=2),
            )
            oh_t = cp.tile([C, S], _BF16, tag="oh")
            nc.sync.dma_start(out=oh_t[:, :], in_=oht[:, :])
            b1_t = cp.tile([128, 1], _F32, tag="b1")
            nc.sync.dma_start(out=b1_t[:, :], in_=b1t[:, :])
            b2_t = cp.tile([64, 1], _F32, tag="b2")
            nc.sync.dma_start(out=b2_t[:, :], in_=b2t[:, :])
            ones = cp.tile([64, 1], _F32, tag="ones")
            nc.vector.memset(ones[:, :], 1.0)

            # per-sample cell+bias rows via one-hot matmul on the idle PE
            # (runs in the shadow of the Q7 library load); duplicated per
            # leg on the way out of PSUM: cells2[:, fb, 2s+leg] = cell[s]
            cells2 = ap.tile([128, 2, L], _BF16, tag="cells2")
            for fb in range(2):
                for q in range(2):
                    psc = pc.tile([128, TN], _F32, tag="psc")
                    nc.tensor.matmul(
                        psc[:, :], cb_t[:, fb, :], oh_t[:, q * TN:(q + 1) * TN],
                        start=True, stop=True,
                    )
                    for leg in range(2):
                        nc.vector.tensor_copy(
                            cells2[:, fb, 2 * q * TN + leg:2 * (q + 1) * TN:2],
                            psc[:, :],
                        )

            # E-row gather: per sample s, rows E[d0], E[d1] at columns
            # 2s, 2s+1; feature-major via transpose mode.
            xts = []
            for g in range(NG_B):
                xt = ap.tile([128, 2, ni_g], _BF16, tag=f"xt{g}")
                nc.gpsimd.dma_gather(
                    xt[:, :, :], etab[:],
                    idx_t[:, g * (ni_g // 16):(g + 1) * (ni_g // 16)],
                    ni_g, ni_g, F,
                    # transpose-mode gathers corrupt with single_packet=False
                    transpose=True, single_packet=True, queue_num=g % NQ,
                )
                xts.append(xt)

            # h0 = relu(E_leg + cellb0[s]) on DVE, bf16, contiguous ops.
            # add and relu are interleaved per wave: the DVE queue is FIFO,
            # so emitting all adds first would park wave 0's relu (which
            # gates the first matmul tile) behind wave 3's drain
            h0 = ap.tile([128, 2, L], _BF16, tag="h0")
            for g in range(NG_B):
                nc.vector.tensor_tensor(
                    out=h0[:, :, g * ni_g:(g + 1) * ni_g],
                    in0=xts[g][:, :, :],
                    in1=cells2[:, :, g * ni_g:(g + 1) * ni_g],
                    op=mybir.AluOpType.add,
                )
                nc.vector.tensor_scalar_max(
                    h0[:, :, g * ni_g:(g + 1) * ni_g],
                    h0[:, :, g * ni_g:(g + 1) * ni_g],
                    0.0,
                )

            h1 = ap.tile([128, L], _BF16, tag="h1")
            h2 = ap.tile([64, L], _F32, tag="h2")
            prod = ap.tile([64, S], _F32, tag="prod")
            out_sb = ap.tile([1, S], _F32, tag="out")
            # layer 1 as two stationary-amortized sweeps (LS of W1 chunk
            # once per c, not once per tile); 4 PSUM banks live at once
            ps1s = [pp1.tile([128, TN], _F32, tag="ps1", name=f"ps1_{i}")
                    for i in range(NT)]
            for c in range(2):
                for nt in range(NT):
                    nc.tensor.matmul(
                        ps1s[nt][:, :], w1_t[:, c, :],
                        h0[:, c, nt * TN:(nt + 1) * TN],
                        start=(c == 0), stop=(c == 1),
                        skip_group_check=True,
                    )
            for nt in range(NT):
                nc.scalar.activation(
                    h1[:, nt * TN:(nt + 1) * TN], ps1s[nt][:, :],
                    mybir.ActivationFunctionType.Relu,
                    bias=b1_t[:, 0:1], scale=1.0,
                )
            for nt in range(NT):
                ps2 = pp.tile([64, TN], _F32, tag="ps2")
                nc.tensor.matmul(
                    ps2[:, :], w2_t[:, :], h1[:, nt * TN:(nt + 1) * TN],
                    start=True, stop=True,
                )
                nc.scalar.activation(
                    h2[:, nt * TN:(nt + 1) * TN], ps2[:, :],
                    mybir.ActivationFunctionType.Identity,
                    bias=b2_t[:, 0:1], scale=1.0,
                )
                # pair product for this tile's 256 samples
                nc.vector.tensor_tensor(
                    out=prod[:, nt * SN:(nt + 1) * SN],
                    in0=h2[:, nt * TN:(nt + 1) * TN:2],
                    in1=h2[:, nt * TN + 1:(nt + 1) * TN:2],
                    op=mybir.AluOpType.mult,
                )
            for nt in range(NT):
                ps3 = pp3.tile([1, SN], _F32, tag="ps3")
                nc.tensor.matmul(
                    ps3[:, :], ones[:, :], prod[:, nt * SN:(nt + 1) * SN],
                    start=True, stop=True,
                )
                nc.vector.tensor_copy(
                    out_sb[:, nt * SN:(nt + 1) * SN], ps3[:, :]
                )
            nc.sync.dma_start(out=y[:, :], in_=out_sb[:, :])
    nc.compile()
    return nc


def _get_kernels():
    if "a" not in _cache:
        _cache["a"] = _build_kernel_a()
    if "b" not in _cache:
        _cache["b"] = _build_kernel_b()
    return _cache["a"], _cache["b"]


def _prep(drug_pairs, cell_lines, drug_targets, W0, b0, W1, b1, W2, b2):
    """Host-side data layout: shard, transpose, cast, build gather indices."""
    dt = np.asarray(drug_targets, dtype=np.int64)                  # [D, T]
    # dedup per row (reference uses .set -> dup targets count once)
    dup = (dt[:, :, None] == dt[:, None, :]) & (
        np.arange(T)[None, :, None] > np.arange(T)[None, None, :]
    )
    idx = np.where(dup.any(-1), ZROW, dt).astype(np.int32)          # [D, T]

    # W0T table: [P+C rows, F] bf16 + zero row + pad
    w0t = np.zeros((TAB_ROWS, F), dtype=ml_dtypes.bfloat16)
    w0t[: P + C] = np.asarray(W0, np.float32).T.astype(ml_dtypes.bfloat16)

    # launch A per-core gather index arrays
    idx_a = []
    for c in range(NCORES):
        rows = np.full((DRUGS_PAD, T), ZROW, np.int32)
        rows[:DRUGS_PER_CORE] = idx[c * DRUGS_PER_CORE:(c + 1) * DRUGS_PER_CORE]
        # flat j = b*4096 + t*128 + p  ->  drug 128b+p, target t
        flat = rows.reshape(4, 128, T).transpose(0, 2, 1).reshape(-1)
        idx_a.append(_wrap_idx(flat))

    # launch B per-core index arrays (E rows per leg) + cell one-hots
    dp = np.asarray(drug_pairs, dtype=np.int64)                     # [B, 2]
    cl = np.asarray(cell_lines, dtype=np.int64)                     # [B]
    e_row = (dp // DRUGS_PER_CORE) * DRUGS_PAD + (dp % DRUGS_PER_CORE)
    idx_b, oh_b = [], []
    for c in range(NCORES):
        sl = slice(c * SAMPLES_PER_CORE, (c + 1) * SAMPLES_PER_CORE)
        idx_b.append(_wrap_idx(e_row[sl].reshape(-1)))
        oh = np.zeros((C, SAMPLES_PER_CORE), dtype=ml_dtypes.bfloat16)
        oh[cl[sl], np.arange(SAMPLES_PER_CORE)] = 1.0
        oh_b.append(oh)

    w1t = np.ascontiguousarray(
        np.asarray(W1, np.float32).T.astype(ml_dtypes.bfloat16))    # [F, H1]
    w2t = np.ascontiguousarray(
        np.asarray(W2, np.float32).T.astype(ml_dtypes.bfloat16))    # [H1, H2]
    b1t = np.asarray(b1, np.float32).reshape(128, 1).copy()
    b2t = np.asarray(b2, np.float32).reshape(64, 1).copy()
    # cell rows with b0 folded in: relu input is E[d] + (W0cell[:,c] + b0)
    celltab = (
        np.asarray(W0, np.float32)[:, P:P + C].T + np.asarray(b0, np.float32)
    ).astype(ml_dtypes.bfloat16)                                    # [C, F]
    return w0t, idx_a, idx_b, oh_b, w1t, w2t, b1t, b2t, celltab


def _run(inputs, trace=False):
    nca, ncb = _get_kernels()
    w0t, idx_a, idx_b, oh_b, w1t, w2t, b1t, b2t, celltab = _prep(**inputs)

    in_a = [{"tab": w0t, "idxs": idx_a[c]} for c in range(NCORES)]
    res_a = run_bass_kernel_spmd(
        nca, in_a, core_ids=list(range(NCORES)), trace=trace)

    e_ext = np.concatenate(
        [res_a.results[c]["e_out"] for c in range(NCORES)], axis=0
    )
    assert e_ext.shape == (E_ROWS, F)

    in_b = [
        {"etab": e_ext, "idxs": idx_b[c], "w1t": w1t, "w2t": w2t,
         "cbt": celltab, "oht": oh_b[c], "b1t": b1t, "b2t": b2t}
        for c in range(NCORES)
    ]
    res_b = run_bass_kernel_spmd(
        ncb, in_b, core_ids=list(range(NCORES)), trace=trace)

    out = np.concatenate(
        [res_b.results[c]["y"].reshape(-1) for c in range(NCORES)]
    ).astype(np.float32)
    times = (res_a.exec_time_ns, res_b.exec_time_ns)
    return out, times


def kernel(**inputs) -> np.ndarray:
    out, _ = _run(inputs, trace=False)
    return out
